# revision 17
# baseline (speedup 1.0000x reference)
"""Dense-MoE (all experts, softmax-gated) Trainium2 kernel — bf16 pipeline.

Math (per token t):
  s1    = x @ [Wd_cat | Wg]              # one K=768 matmul -> [64 | 8 logits]
  expu  = exp(s1[64:72] + bg)            # unnormalized gates            [8]
  g64   = e8 @ expu                      # expanded per-expert gates     [64]
  h1b   = s1[0:64] + bd
  s2    = Wm_blockdiag @ h1b             # K=64 matmul
  s3in  = [(s2 + bm) * g64 ; expu]       # [72]
  o     = s3in @ [[1|1|Wu_cat],[..;bu]]  # K=72 matmul; cols 0,1 = Z
  out   = o[2:] / o[0]                   # softmax normalization at the end,
                                         # fused into the psum->sbuf cast

All tensors bf16 (x cast host-side, out stored bf16 and cast back on host;
psum accumulation fp32). x is loaded pre-transposed into SBUF via the DMA
xbar (16x128-tile transpose), so the PE does no transposes at all.

Sharding: data-parallel over tokens, 8 cores, weights replicated.
"""

import numpy as np

B, S, D, E, R = 8, 4096, 768, 8, 8
NCORES = 8
T_CORE = B * S // NCORES          # 4096 tokens per core
TILE_T = 512                      # tokens per compute tile
N_TILES = T_CORE // TILE_T        # 8
EW = E * R                        # 64
KW = EW + E                       # 72
KC = D // 128                    # 6 contraction chunks for stage 1
JC = TILE_T // 128                # 4 token chunks of 128 per tile

# packed bf16 weight column offsets
O_W1 = 0
O_E8 = O_W1 + KC * KW             # 432
O_WM = O_E8 + EW                  # 496
O_W3 = O_WM + EW                  # 560
NW = O_W3 + 2 + D                 # 1330

_CACHE = {}


def _build_and_compile():
    """Build the Bass/Tile program once. Returns compiled nc."""
    from contextlib import ExitStack

    import concourse.bass as bass
    import concourse.tile as tile
    from concourse import bacc, mybir

    f32 = mybir.dt.float32
    bf16 = mybir.dt.bfloat16
    AF = mybir.ActivationFunctionType
    ALU = mybir.AluOpType

    nc = bacc.Bacc("TRN2", target_bir_lowering=False, debug=False, num_devices=NCORES)

    x_d = nc.dram_tensor("x", [T_CORE, D], bf16, kind="ExternalInput").ap()
    wp_d = nc.dram_tensor("wpack", [128, NW], bf16, kind="ExternalInput").ap()
    wb_d = nc.dram_tensor("wb32", [128, 3], f32, kind="ExternalInput").ap()
    out_d = nc.dram_tensor("out", [T_CORE, D], bf16, kind="ExternalOutput").ap()

    # output: partition p of tile i, chunk j holds token i*512 + j*128 + p
    out_v = out_d.rearrange("(i j p) d -> i p j d", j=JC, p=128)

    with tile.TileContext(nc) as tc, ExitStack() as ctx:
        const = ctx.enter_context(tc.tile_pool(name="const", bufs=1))
        xin = ctx.enter_context(tc.tile_pool(name="xin", bufs=4))
        work = ctx.enter_context(tc.tile_pool(name="work", bufs=2))
        outp = ctx.enter_context(tc.tile_pool(name="outp", bufs=4))
        small = ctx.enter_context(tc.tile_pool(name="small", bufs=4))
        # PSUM (8 banks): s1 2 + s2 1 + g64 1 + s3 4
        s1p = ctx.enter_context(tc.tile_pool(name="s1p", bufs=2, space="PSUM"))
        s2p = ctx.enter_context(tc.tile_pool(name="s2p", bufs=1, space="PSUM"))
        g64p = ctx.enter_context(tc.tile_pool(name="g64p", bufs=1, space="PSUM"))
        s3ap = ctx.enter_context(tc.tile_pool(name="s3ap", bufs=4, space="PSUM"))

        xts, s3ins, outsbs = {}, {}, {}

        def load(i):
            xt = xin.tile([128, KC * TILE_T], bf16, name="xt", tag="x")
            nc.sync.dma_start(
                xt[:].rearrange("p (c t) -> p c t", c=KC),
                x_d[i * TILE_T:(i + 1) * TILE_T, :],
                transpose=True,
            )
            xts[i] = xt

        # tile-0 load first on the SP ring (HWDGE serializes transfers, so
        # the big load wins the race), weights right behind it.
        load(0)
        wp = const.tile([128, NW], bf16, name="wp")
        nc.sync.dma_start(wp[:], wp_d)
        wb = const.tile([128, 3], f32, name="wb")
        nc.sync.dma_start(wb[:], wb_d)

        w1_sb = wp[:, O_W1:O_W1 + KC * KW]
        e8_sb = wp[EW:KW, O_E8:O_E8 + EW]
        wm_sb = wp[0:EW, O_WM:O_WM + EW]
        w3_sb = wp[0:KW, O_W3:O_W3 + 2 + D]
        bd_sb = wb[0:EW, 0:1]
        bg_sb = wb[EW:KW, 1:2]
        bm_sb = wb[0:EW, 2:3]

        # PE pre-warm on garbage zeros (no DMA dependency) so the clock is
        # ramped when tile 0 lands.
        warm_src = const.tile([128, TILE_T], bf16, name="warm_src")
        nc.gpsimd.memset(warm_src[:], 0.0)
        warm_ps = s1p.tile([128, TILE_T], f32, name="warm_ps", tag="s1")
        for _k in range(12):
            nc.tensor.matmul(
                warm_ps[:], warm_src[:, 0:128], warm_src[:], start=True, stop=True
            )

        def front(i):
            """stage-1 matmuls + psum-freeing epilogues (exp, bias-add)."""
            xt = xts[i]
            s1 = s1p.tile([KW, TILE_T], f32, name="s1", tag="s1")
            for c in range(KC):
                nc.tensor.matmul(
                    s1[:],
                    w1_sb[:, c * KW:(c + 1) * KW],
                    xt[:, c * TILE_T:(c + 1) * TILE_T],
                    start=(c == 0),
                    stop=(c == KC - 1),
                )
            s3in = work.tile([KW, TILE_T], bf16, name="s3in", tag="s3in")
            nc.scalar.activation(s3in[EW:KW, :], s1[EW:KW, :], AF.Exp, bias=bg_sb)
            h1b = work.tile([EW, TILE_T], bf16, name="h1b", tag="h1b")
            nc.vector.tensor_scalar_add(h1b[:], s1[0:EW, :], bd_sb)
            return s3in, h1b

        def mid(i, s3in, h1b):
            """gating chain -> s3in ready."""
            g64 = g64p.tile([EW, TILE_T], f32, name="g64", tag="g64")
            nc.tensor.matmul(g64[:], e8_sb, s3in[EW:KW, :], start=True, stop=True)
            g64u = work.tile([EW, TILE_T], bf16, name="g64u", tag="g64u")
            nc.scalar.copy(g64u[:], g64[:])
            s2 = s2p.tile([EW, TILE_T], f32, name="s2", tag="s2")
            nc.tensor.matmul(s2[:], wm_sb, h1b[:], start=True, stop=True)
            nc.vector.scalar_tensor_tensor(
                s3in[0:EW, :], s2[:], bm_sb, g64u[:], op0=ALU.add, op1=ALU.mult
            )
            s3ins[i] = s3in
            out_sb = outp.tile([128, JC * D], bf16, name="out_sb", tag="out")
            outsbs[i] = out_sb

        def back(i):
            """stage-3 matmuls, fused normalize + psum->sbuf casts, store."""
            s3in, out_sb = s3ins.pop(i), outsbs.pop(i)
            for j in range(JC):
                lhsT = s3in[:, j * 128:(j + 1) * 128]
                s3a = s3ap.tile([128, 386], f32, name="s3a", tag="s3")
                nc.tensor.matmul(
                    s3a[:], lhsT, w3_sb[:, 0:386], start=True, stop=True
                )
                s3b = s3ap.tile([128, 384], f32, name="s3b", tag="s3")
                nc.tensor.matmul(
                    s3b[:], lhsT, w3_sb[:, 386:2 + D], start=True, stop=True
                )
                rc = small.tile([128, 1], f32, name="rc", tag="rc")
                nc.vector.reciprocal(rc[:], s3a[:, 0:1])
                oa = out_sb[:, j * D:j * D + 384]
                ob = out_sb[:, j * D + 384:(j + 1) * D]
                nc.scalar.mul(oa, s3a[:, 2:386], rc[:])
                nc.vector.tensor_scalar_mul(ob, s3b[:], rc[:])
            nc.sync.dma_start(
                out_v[i, :, :, :], out_sb[:].rearrange("p (j d) -> p j d", j=JC)
            )
            xts.pop(i)

        # software pipeline: loads prefetch 2 ahead. Emission order per
        # iteration is mid(i) -> front(i+1) -> back(i): the Act queue then
        # runs g64u(i) before exp(i+1), so tile i's gating chain completes
        # while the PE streams s1(i+1), and s3(i) starts with no bubble.
        load(1)
        fr = {0: front(0)}
        for i in range(N_TILES):
            if i + 2 < N_TILES:
                load(i + 2)
            mid(i, *fr.pop(i))
            if i + 1 < N_TILES:
                fr[i + 1] = front(i + 1)
            back(i)

    nc.compile()
    return nc


def _pack_host_inputs(Wd, bd, Wm, bm, Wu, bu, Wg, bg):
    """Repack the tiny weights into on-chip layouts (host-side, ~200KB)."""
    import ml_dtypes

    f = np.float32
    bf = ml_dtypes.bfloat16
    W1 = np.concatenate(
        [np.ascontiguousarray(Wd.transpose(1, 0, 2)).reshape(D, EW), Wg], axis=1
    ).astype(f)                                   # [768, 72]
    w1p = np.ascontiguousarray(
        W1.reshape(KC, 128, KW).transpose(1, 0, 2)
    ).reshape(128, KC * KW)                       # [128, 432]; chunk c at cols c*72

    e8 = np.kron(np.eye(E, dtype=f), np.ones((1, R), f))   # [8, 64]

    wmbd = np.zeros((EW, EW), f)
    for e in range(E):
        wmbd[e * R:(e + 1) * R, e * R:(e + 1) * R] = Wm[e]

    w3e = np.zeros((KW, 2 + D), f)
    w3e[EW:, 0] = 1.0
    w3e[EW:, 1] = 1.0
    w3e[:EW, 2:] = Wu.reshape(EW, D)
    w3e[EW:, 2:] = bu

    wpack = np.zeros((128, NW), f)
    wpack[:, O_W1:O_W1 + KC * KW] = w1p
    wpack[EW:KW, O_E8:O_E8 + EW] = e8
    wpack[0:EW, O_WM:O_WM + EW] = wmbd
    wpack[0:KW, O_W3:O_W3 + 2 + D] = w3e

    wb32 = np.zeros((128, 3), f)
    wb32[0:EW, 0] = bd.reshape(EW)
    wb32[EW:KW, 1] = bg.reshape(E)
    wb32[0:EW, 2] = bm.reshape(EW)
    return {"wpack": wpack.astype(bf), "wb32": wb32}


def _run(inputs, trace=False, **kw):
    import ml_dtypes

    from concourse import bass_utils

    if "nc" not in _CACHE:
        _CACHE["nc"] = _build_and_compile()
    nc = _CACHE["nc"]

    bf = ml_dtypes.bfloat16
    x = np.ascontiguousarray(
        np.asarray(inputs["x"], dtype=np.float32).reshape(B * S, D).astype(bf)
    )
    w = _pack_host_inputs(
        *(np.asarray(inputs[k], dtype=np.float32)
          for k in ["Wd", "bd", "Wm", "bm", "Wu", "bu", "Wg", "bg"])
    )
    in_maps = [
        {"x": np.ascontiguousarray(x[i * T_CORE:(i + 1) * T_CORE]), **w}
        for i in range(NCORES)
    ]
    res = bass_utils.run_bass_kernel_spmd(
        nc, in_maps, core_ids=list(range(NCORES)), trace=trace, **kw
    )
    out = np.concatenate(
        [np.asarray(res.results[i]["out"]) for i in range(NCORES)], axis=0
    ).astype(np.float32).reshape(B, S, D)
    return out, res


def kernel(**inputs) -> np.ndarray:
    out, _ = _run(inputs)
    return out


# revision 18
# speedup vs baseline: 1.0436x; 1.0436x over previous
"""Dense-MoE (all experts, softmax-gated) Trainium2 kernel — bf16 pipeline.

Math (per token t):
  s1    = x @ [Wd_cat | Wg]              # one K=768 matmul -> [64 | 8 logits]
  expu  = exp(s1[64:72] + bg)            # unnormalized gates            [8]
  g64   = e8 @ expu                      # expanded per-expert gates     [64]
  h1b   = s1[0:64] + bd
  s2    = Wm_blockdiag @ h1b             # K=64 matmul
  s3in  = [(s2 + bm) * g64 ; expu]       # [72]
  o     = s3in @ [[1|1|Wu_cat],[..;bu]]  # K=72 matmul; cols 0,1 = Z
  out   = o[2:] / o[0]                   # softmax normalization at the end,
                                         # fused into the psum->sbuf cast

All tensors bf16 (x cast host-side, out stored bf16 and cast back on host;
psum accumulation fp32). x is loaded pre-transposed into SBUF via the DMA
xbar (16x128-tile transpose), so the PE does no transposes at all.

Sharding: data-parallel over tokens, 8 cores, weights replicated.
"""

import numpy as np

B, S, D, E, R = 8, 4096, 768, 8, 8
NCORES = 8
T_CORE = B * S // NCORES          # 4096 tokens per core
TILE_T = 512                      # tokens per compute tile
N_TILES = T_CORE // TILE_T        # 8
EW = E * R                        # 64
KW = EW + E                       # 72
KC = D // 128                    # 6 contraction chunks for stage 1
JC = TILE_T // 128                # 4 token chunks of 128 per tile

# packed bf16 weight column offsets
O_W1 = 0
O_E8 = O_W1 + KC * KW             # 432
O_WM = O_E8 + EW                  # 496
O_W3 = O_WM + EW                  # 560
NW = O_W3 + 2 + D                 # 1330

_CACHE = {}


def _build_and_compile():
    """Build the Bass/Tile program once. Returns compiled nc."""
    from contextlib import ExitStack

    import concourse.bass as bass
    import concourse.tile as tile
    from concourse import bacc, mybir

    f32 = mybir.dt.float32
    bf16 = mybir.dt.bfloat16
    AF = mybir.ActivationFunctionType
    ALU = mybir.AluOpType

    nc = bacc.Bacc("TRN2", target_bir_lowering=False, debug=False, num_devices=NCORES)

    x_d = nc.dram_tensor("x", [T_CORE, D], bf16, kind="ExternalInput").ap()
    wp_d = nc.dram_tensor("wpack", [128, NW], bf16, kind="ExternalInput").ap()
    wb_d = nc.dram_tensor("wb32", [128, 3], f32, kind="ExternalInput").ap()
    out_d = nc.dram_tensor("out", [T_CORE, D], bf16, kind="ExternalOutput").ap()

    # output: partition p of tile i, chunk j holds token i*512 + j*128 + p
    out_v = out_d.rearrange("(i j p) d -> i p j d", j=JC, p=128)

    with tile.TileContext(nc) as tc, ExitStack() as ctx:
        const = ctx.enter_context(tc.tile_pool(name="const", bufs=1))
        xin = ctx.enter_context(tc.tile_pool(name="xin", bufs=3))
        work = ctx.enter_context(tc.tile_pool(name="work", bufs=2))
        outp = ctx.enter_context(tc.tile_pool(name="outp", bufs=3))
        small = ctx.enter_context(tc.tile_pool(name="small", bufs=4))
        # PSUM (8 banks): s1 2 + s2 1 + g64 1 + s3 4
        s1p = ctx.enter_context(tc.tile_pool(name="s1p", bufs=2, space="PSUM"))
        s2p = ctx.enter_context(tc.tile_pool(name="s2p", bufs=1, space="PSUM"))
        g64p = ctx.enter_context(tc.tile_pool(name="g64p", bufs=1, space="PSUM"))
        s3ap = ctx.enter_context(tc.tile_pool(name="s3ap", bufs=4, space="PSUM"))

        xts, s3ins, outsbs = {}, {}, {}

        def load(i):
            xt = xin.tile([128, KC * TILE_T], bf16, name="xt", tag="x")
            nc.sync.dma_start(
                xt[:].rearrange("p (c t) -> p c t", c=KC),
                x_d[i * TILE_T:(i + 1) * TILE_T, :],
                transpose=True,
            )
            xts[i] = xt

        # tile-0 load first on the SP ring (HWDGE serializes transfers, so
        # the big load wins the race), weights right behind it.
        load(0)
        wp = const.tile([128, NW], bf16, name="wp")
        nc.sync.dma_start(wp[:], wp_d)
        wb = const.tile([128, 3], f32, name="wb")
        nc.sync.dma_start(wb[:], wb_d)

        w1_sb = wp[:, O_W1:O_W1 + KC * KW]
        e8_sb = wp[EW:KW, O_E8:O_E8 + EW]
        wm_sb = wp[0:EW, O_WM:O_WM + EW]
        w3_sb = wp[0:KW, O_W3:O_W3 + 2 + D]
        bd_sb = wb[0:EW, 0:1]
        bg_sb = wb[EW:KW, 1:2]
        bm_sb = wb[0:EW, 2:3]

        # PE pre-warm on garbage zeros (no DMA dependency) so the clock is
        # ramped when tile 0 lands.
        warm_src = const.tile([128, TILE_T], bf16, name="warm_src")
        nc.gpsimd.memset(warm_src[:], 0.0)
        warm_ps = s1p.tile([128, TILE_T], f32, name="warm_ps", tag="s1")
        for _k in range(12):
            nc.tensor.matmul(
                warm_ps[:], warm_src[:, 0:128], warm_src[:], start=True, stop=True
            )

        def front(i):
            """stage-1 matmuls + psum-freeing epilogues (exp, bias-add)."""
            xt = xts[i]
            s1 = s1p.tile([KW, TILE_T], f32, name="s1", tag="s1")
            for c in range(KC):
                nc.tensor.matmul(
                    s1[:],
                    w1_sb[:, c * KW:(c + 1) * KW],
                    xt[:, c * TILE_T:(c + 1) * TILE_T],
                    start=(c == 0),
                    stop=(c == KC - 1),
                )
            s3in = work.tile([KW, TILE_T], bf16, name="s3in", tag="s3in")
            nc.scalar.activation(s3in[EW:KW, :], s1[EW:KW, :], AF.Exp, bias=bg_sb)
            h1b = work.tile([EW, TILE_T], bf16, name="h1b", tag="h1b")
            nc.vector.tensor_scalar_add(h1b[:], s1[0:EW, :], bd_sb)
            return s3in, h1b

        def mid(i, s3in, h1b):
            """gating chain -> s3in ready."""
            g64 = g64p.tile([EW, TILE_T], f32, name="g64", tag="g64")
            nc.tensor.matmul(g64[:], e8_sb, s3in[EW:KW, :], start=True, stop=True)
            g64u = work.tile([EW, TILE_T], bf16, name="g64u", tag="g64u")
            nc.scalar.copy(g64u[:], g64[:])
            s2 = s2p.tile([EW, TILE_T], f32, name="s2", tag="s2")
            nc.tensor.matmul(s2[:], wm_sb, h1b[:], start=True, stop=True)
            nc.vector.scalar_tensor_tensor(
                s3in[0:EW, :], s2[:], bm_sb, g64u[:], op0=ALU.add, op1=ALU.mult
            )
            s3ins[i] = s3in
            out_sb = outp.tile([128, JC * D], bf16, name="out_sb", tag="out")
            outsbs[i] = out_sb

        def back(i):
            """stage-3 matmuls, fused normalize + psum->sbuf casts, store."""
            s3in, out_sb = s3ins.pop(i), outsbs.pop(i)
            for j in range(JC):
                lhsT = s3in[:, j * 128:(j + 1) * 128]
                s3a = s3ap.tile([128, 386], f32, name="s3a", tag="s3")
                nc.tensor.matmul(
                    s3a[:], lhsT, w3_sb[:, 0:386], start=True, stop=True
                )
                s3b = s3ap.tile([128, 384], f32, name="s3b", tag="s3")
                nc.tensor.matmul(
                    s3b[:], lhsT, w3_sb[:, 386:2 + D], start=True, stop=True
                )
                rc = small.tile([128, 1], f32, name="rc", tag="rc")
                nc.vector.reciprocal(rc[:], s3a[:, 0:1])
                oa = out_sb[:, j * D:j * D + 384]
                ob = out_sb[:, j * D + 384:(j + 1) * D]
                nc.scalar.mul(oa, s3a[:, 2:386], rc[:])
                nc.vector.tensor_scalar_mul(ob, s3b[:], rc[:])
            nc.sync.dma_start(
                out_v[i, :, :, :], out_sb[:].rearrange("p (j d) -> p j d", j=JC)
            )
            xts.pop(i)

        # software pipeline: loads prefetch 2 ahead. Emission order per
        # iteration is mid(i) -> front(i+1) -> back(i): the Act queue then
        # runs g64u(i) before exp(i+1), so tile i's gating chain completes
        # while the PE streams s1(i+1), and s3(i) starts with no bubble.
        load(1)
        fr = {0: front(0)}
        for i in range(N_TILES):
            if i + 2 < N_TILES:
                load(i + 2)
            mid(i, *fr.pop(i))
            if i + 1 < N_TILES:
                fr[i + 1] = front(i + 1)
            back(i)

    nc.compile()
    return nc


def _pack_host_inputs(Wd, bd, Wm, bm, Wu, bu, Wg, bg):
    """Repack the tiny weights into on-chip layouts (host-side, ~200KB)."""
    import ml_dtypes

    f = np.float32
    bf = ml_dtypes.bfloat16
    W1 = np.concatenate(
        [np.ascontiguousarray(Wd.transpose(1, 0, 2)).reshape(D, EW), Wg], axis=1
    ).astype(f)                                   # [768, 72]
    w1p = np.ascontiguousarray(
        W1.reshape(KC, 128, KW).transpose(1, 0, 2)
    ).reshape(128, KC * KW)                       # [128, 432]; chunk c at cols c*72

    e8 = np.kron(np.eye(E, dtype=f), np.ones((1, R), f))   # [8, 64]

    wmbd = np.zeros((EW, EW), f)
    for e in range(E):
        wmbd[e * R:(e + 1) * R, e * R:(e + 1) * R] = Wm[e]

    w3e = np.zeros((KW, 2 + D), f)
    w3e[EW:, 0] = 1.0
    w3e[EW:, 1] = 1.0
    w3e[:EW, 2:] = Wu.reshape(EW, D)
    w3e[EW:, 2:] = bu

    wpack = np.zeros((128, NW), f)
    wpack[:, O_W1:O_W1 + KC * KW] = w1p
    wpack[EW:KW, O_E8:O_E8 + EW] = e8
    wpack[0:EW, O_WM:O_WM + EW] = wmbd
    wpack[0:KW, O_W3:O_W3 + 2 + D] = w3e

    wb32 = np.zeros((128, 3), f)
    wb32[0:EW, 0] = bd.reshape(EW)
    wb32[EW:KW, 1] = bg.reshape(E)
    wb32[0:EW, 2] = bm.reshape(EW)
    return {"wpack": wpack.astype(bf), "wb32": wb32}


def _run(inputs, trace=False, **kw):
    import ml_dtypes

    from concourse import bass_utils

    if "nc" not in _CACHE:
        _CACHE["nc"] = _build_and_compile()
    nc = _CACHE["nc"]

    bf = ml_dtypes.bfloat16
    x = np.ascontiguousarray(
        np.asarray(inputs["x"], dtype=np.float32).reshape(B * S, D).astype(bf)
    )
    w = _pack_host_inputs(
        *(np.asarray(inputs[k], dtype=np.float32)
          for k in ["Wd", "bd", "Wm", "bm", "Wu", "bu", "Wg", "bg"])
    )
    in_maps = [
        {"x": np.ascontiguousarray(x[i * T_CORE:(i + 1) * T_CORE]), **w}
        for i in range(NCORES)
    ]
    res = bass_utils.run_bass_kernel_spmd(
        nc, in_maps, core_ids=list(range(NCORES)), trace=trace, **kw
    )
    out = np.concatenate(
        [np.asarray(res.results[i]["out"]) for i in range(NCORES)], axis=0
    ).astype(np.float32).reshape(B, S, D)
    return out, res


def kernel(**inputs) -> np.ndarray:
    out, _ = _run(inputs)
    return out
